# revision 33
# baseline (speedup 1.0000x reference)
"""Trainium2 Bass kernel for a 2-layer GATv2 GNN (nn_AttGCNN).

Strategy (8 NeuronCores, dst-node graph partition):
  - Nodes sharded by dst across 8 cores (6250 each). Edges (incl. self
    loops) sorted by dst, grouped into per-core 128-dst-node blocks and
    padded to 128-edge tiles (uniform tile counts across cores -> SPMD).
  - Layer 1: every core computes the full xl = x@Wl1 table (gather
    source table) and its local xr rows. Edge pass per block: indirect
    DMA gathers xl[src] rows; a one-hot matmul expands xr[dst] per edge
    and adds xl via an identity matmul; LeakyReLU (ScalarE) + att dot
    (TensorE). The segment softmax uses exp(alpha) directly -- alpha is
    O(10) for this model so no segment-max subtraction is needed; the
    result is identical up to rounding. Weighted messages plus a
    weight-sum column are aggregated per 128-dst block with a one-hot
    matmul accumulated in PSUM, then normalized.
  - Layer 2 (out dim 4): xl2 = h@Wl2 computed locally, AllGathered
    (100KB), same edge pass with 4-wide features, fused final softmax.
"""
import os
import sys
import numpy as np

sys.path.insert(0, "/opt/trn_rl_repo")

N = 50000
NC = 8
NLOC = N // NC            # 6250
NBLK = (NLOC + 127) // 128  # 49
NPAD = 50048              # 391 * 128
GANG = 8                  # tiles per gather instruction
GRP = 4                   # tiles per fused DVE/ACT group
MAXTB = 32                # max tiles per block supported

_EXEC_NS = {"v": None}


def _preprocess(edge_index):
    src = np.concatenate([np.asarray(edge_index[0], dtype=np.int64),
                          np.arange(N, dtype=np.int64)])
    dst = np.concatenate([np.asarray(edge_index[1], dtype=np.int64),
                          np.arange(N, dtype=np.int64)])
    order = np.argsort(dst, kind="stable")
    src, dst = src[order], dst[order]
    deg = np.bincount(dst, minlength=N)
    node_start = np.zeros(N + 1, dtype=np.int64)
    np.cumsum(deg, out=node_start[1:])
    cnt = np.zeros((NC, NBLK), dtype=np.int64)
    for c in range(NC):
        for b in range(NBLK):
            lo = c * NLOC + b * 128
            hi = min(c * NLOC + (b + 1) * 128, (c + 1) * NLOC)
            cnt[c, b] = deg[lo:hi].sum()
    T_b = np.maximum(1, np.ceil(cnt / 128).astype(np.int64).max(axis=0))
    assert T_b.max() <= MAXTB, T_b.max()
    T = int(T_b.sum())
    tile0 = np.concatenate([[0], np.cumsum(T_b)]).astype(np.int64)
    src_off = np.zeros((NC, 128, T), dtype=np.int32)
    dst_col = np.full((NC, 128, T), -1.0, dtype=np.float32)
    dst_row = np.full((NC, 1, 128 * T), -1.0, dtype=np.float32)
    for c in range(NC):
        for b in range(NBLK):
            lo = c * NLOC + b * 128
            hi = min(c * NLOC + (b + 1) * 128, (c + 1) * NLOC)
            e0, e1 = node_start[lo], node_start[hi]
            ne = int(e1 - e0)
            npad = int(T_b[b]) * 128
            sp = np.zeros(npad, dtype=np.int32)
            sp[:ne] = src[e0:e1]
            dp = np.full(npad, -1.0, dtype=np.float32)
            dp[:ne] = (dst[e0:e1] - lo).astype(np.float32)
            t0, t1 = int(tile0[b]), int(tile0[b + 1])
            src_off[c, :, t0:t1] = sp.reshape(-1, 128).T
            dst_col[c, :, t0:t1] = dp.reshape(-1, 128).T
            dst_row[c, 0, t0 * 128:t1 * 128] = dp
    return src_off, dst_col, dst_row, T_b, tile0, T


def _hoist_multi_waits(nc, mybir):
    """This walrus build encodes at most ONE sync wait per TPB instruction;
    hoist extra waits onto standalone NOPs on the same engine stream."""
    for f in nc.m.functions:
        for bb in f.blocks:
            out = []
            for inst in bb.instructions:
                si = inst.sync_info
                waits = list(si.on_wait) if si is not None else []
                if len(waits) > 1:
                    for w in waits[:-1]:
                        nop = mybir.InstNoOp(
                            name=nc.get_next_instruction_name(), ins=[], outs=[])
                        nop.engine = inst.engine
                        nop.sync_info = mybir.SyncInfo(on_wait=[w], on_update=[])
                        out.append(nop)
                    inst.sync_info = mybir.SyncInfo(
                        on_wait=[waits[-1]], on_update=list(si.on_update))
                out.append(inst)
            bb.instructions = out


def _build_program(T_b, tile0, T, use_bias, no_collective=False):
    import concourse.bass as bass
    import concourse.mybir as mybir
    import concourse.tile as tile

    fp32 = mybir.dt.float32
    i32 = mybir.dt.int32
    AF = mybir.ActivationFunctionType
    OP = mybir.AluOpType

    nc = bass.Bass(num_swdge_queues=4)
    # ---- external inputs -------------------------------------------------
    xT = nc.dram_tensor("xT", [128, NPAD], fp32, kind="ExternalInput")
    xT_loc = nc.dram_tensor("xT_loc", [128, NBLK * 128], fp32, kind="ExternalInput")
    Wl1 = nc.dram_tensor("Wl1", [128, 128], fp32, kind="ExternalInput")
    Wr1 = nc.dram_tensor("Wr1", [128, 128], fp32, kind="ExternalInput")
    att1 = nc.dram_tensor("att1", [128, 2], fp32, kind="ExternalInput")
    Wl2 = nc.dram_tensor("Wl2", [128, 4], fp32, kind="ExternalInput")
    Wr2 = nc.dram_tensor("Wr2", [128, 4], fp32, kind="ExternalInput")
    att2 = nc.dram_tensor("att2", [4, 1], fp32, kind="ExternalInput")
    bl1 = nc.dram_tensor("bl1", [128, 128], fp32, kind="ExternalInput")
    br1 = nc.dram_tensor("br1", [128, 128], fp32, kind="ExternalInput")
    bias1r = nc.dram_tensor("bias1r", [128, 128], fp32, kind="ExternalInput")
    bl2 = nc.dram_tensor("bl2", [128, 4], fp32, kind="ExternalInput")
    br2 = nc.dram_tensor("br2", [128, 4], fp32, kind="ExternalInput")
    bias2r = nc.dram_tensor("bias2r", [128, 4], fp32, kind="ExternalInput")
    ones_row = nc.dram_tensor("ones_row", [1, 128], fp32, kind="ExternalInput")
    ident = nc.dram_tensor("ident", [128, 128], fp32, kind="ExternalInput")
    iota_col = nc.dram_tensor("iota_col", [128, 1], fp32, kind="ExternalInput")
    iota_rep = nc.dram_tensor("iota_rep", [128, 128], fp32, kind="ExternalInput")
    src_off = nc.dram_tensor("src_off", [128, T], i32, kind="ExternalInput")
    dst_col = nc.dram_tensor("dst_col", [128, T], fp32, kind="ExternalInput")
    dst_row = nc.dram_tensor("dst_row", [1, 128 * T], fp32, kind="ExternalInput")
    out = nc.dram_tensor("out", [NLOC, 4], fp32, kind="ExternalOutput")

    with tile.TileContext(nc) as tc:
        with (
            tc.tile_pool(name="dram", bufs=1, space="DRAM") as dram,
            tc.tile_pool(name="const", bufs=1) as cpool,
            tc.tile_pool(name="sbuf", bufs=3) as sb,
            tc.tile_pool(name="psum", bufs=2, space="PSUM") as ps,
            tc.tile_pool(name="psum1", bufs=1, space="PSUM") as ps1,
        ):
            xl_dram = dram.tile([NPAD, 128], fp32, tag="xl_dram")
            xr_dram = dram.tile([NBLK * 128, 128], fp32, tag="xr_dram")
            hT_dram = dram.tile([128, NBLK * 128], fp32, tag="hT_dram")
            xl2_loc = dram.tile([NLOC, 4], fp32, tag="xl2_loc")
            xr2_dram = dram.tile([NBLK * 128, 4], fp32, tag="xr2_dram")
            xl2_full = dram.tile([N, 4], fp32, tag="xl2_full")

            # constants
            Wl1_sb = cpool.tile([128, 128], fp32, tag="Wl1")
            Wr1_sb = cpool.tile([128, 128], fp32, tag="Wr1")
            att1_sb = cpool.tile([128, 2], fp32, tag="att1")
            Wl2_sb = cpool.tile([128, 4], fp32, tag="Wl2")
            Wr2_sb = cpool.tile([128, 4], fp32, tag="Wr2")
            att2_sb = cpool.tile([4, 1], fp32, tag="att2")
            bl1_sb = cpool.tile([128, 128], fp32, tag="bl1")
            br1_sb = cpool.tile([128, 128], fp32, tag="br1")
            bias1_sb = cpool.tile([128, 128], fp32, tag="bias1")
            bl2_sb = cpool.tile([128, 4], fp32, tag="bl2")
            br2_sb = cpool.tile([128, 4], fp32, tag="br2")
            bias2_sb = cpool.tile([128, 4], fp32, tag="bias2")
            ones_sb = cpool.tile([1, 128], fp32, tag="ones")
            ident_sb = cpool.tile([128, 128], fp32, tag="ident")
            ic_sb = cpool.tile([128, 1], fp32, tag="ic")
            ir_sb = cpool.tile([128, 128], fp32, tag="ir")
            for dst_t, src_t in [
                (Wl1_sb, Wl1), (Wr1_sb, Wr1), (att1_sb, att1), (Wl2_sb, Wl2),
                (Wr2_sb, Wr2), (att2_sb, att2), (bl1_sb, bl1), (br1_sb, br1),
                (bias1_sb, bias1r), (bl2_sb, bl2), (br2_sb, br2),
                (bias2_sb, bias2r), (ones_sb, ones_row), (ident_sb, ident),
                (ic_sb, iota_col), (ir_sb, iota_rep),
            ]:
                nc.sync.dma_start(out=dst_t[:], in_=src_t[:])

            # ---- phase A/B: xl_full, xr_loc ------------------------------
            def proj(src_ap, n_tiles, W_sb, b_sb, dst_dram, D2=128):
                for g0 in range(0, n_tiles, GRP):
                    n = min(GRP, n_tiles - g0)
                    xt = sb.tile([128, GRP * 128], fp32, tag="xt")
                    nc.sync.dma_start(
                        out=xt[:, : n * 128],
                        in_=src_ap[:, g0 * 128:(g0 + n) * 128])
                    pp = ps.tile([128, GRP * D2], fp32, tag="eT")
                    for j in range(n):
                        nc.tensor.matmul(pp[:, j * D2:(j + 1) * D2],
                                         xt[:, j * 128:(j + 1) * 128], W_sb[:],
                                         start=True, stop=True)
                    xs = sb.tile([128, GRP * D2], fp32, tag="xs")
                    nc.scalar.copy(xs[:, : n * D2], pp[:, : n * D2])
                    if use_bias:
                        nc.vector.tensor_tensor(
                            xs[:, : n * D2].rearrange("p (t d) -> p t d", t=n),
                            xs[:, : n * D2].rearrange("p (t d) -> p t d", t=n),
                            b_sb[:, None, :].to_broadcast([128, n, D2]),
                            OP.add)
                    for j in range(n):
                        r0 = (g0 + j) * 128
                        nc.sync.dma_start(
                            out=dst_dram[r0:r0 + 128, :],
                            in_=xs[:, j * D2:(j + 1) * D2])

            proj(xT[:], NPAD // 128, Wl1_sb, bl1_sb, xl_dram)
            proj(xT_loc[:], NBLK, Wr1_sb, br1_sb, xr_dram)

            # ---- edge pass (shared for both layers) ----------------------
            def edge_pass(D, H, xtab_ap, xr_dram_t, att_sb, block_epilogue):
                CH = D // H
                for b in range(NBLK):
                    t0, t1 = int(tile0[b]), int(tile0[b + 1])
                    Tb = t1 - t0
                    so_sb = sb.tile([128, MAXTB], i32, tag="so")
                    dc_sb = sb.tile([128, MAXTB], fp32, tag="dc")
                    dr_sb = sb.tile([1, MAXTB * 128], fp32, tag="dr")
                    nc.sync.dma_start(out=so_sb[:, :Tb], in_=src_off[:, t0:t1])
                    nc.sync.dma_start(out=dc_sb[:, :Tb], in_=dst_col[:, t0:t1])
                    nc.sync.dma_start(out=dr_sb[:, :Tb * 128],
                                      in_=dst_row[:, t0 * 128:t1 * 128])
                    xr_sb = sb.tile([128, D], fp32, tag="xr")
                    nc.sync.dma_start(out=xr_sb[:],
                                      in_=xr_dram_t[b * 128:(b + 1) * 128, :])
                    agg_ps = ps.tile([128, D + H], fp32, tag="agg")
                    gath = []
                    for q0 in range(0, Tb, GANG):
                        qn = min(GANG, Tb - q0)
                        xg = sb.tile([128, GANG, D], fp32, tag="xg")
                        for q in range(qn):
                            gi = nc.gpsimd.indirect_dma_start(
                                out=xg[:, q, :],
                                out_offset=None,
                                in_=xtab_ap,
                                in_offset=bass.IndirectOffsetOnAxis(
                                    ap=so_sb[:, q0 + q:q0 + q + 1], axis=0),
                            )
                            qi = (q0 + q) % 4
                            gi.ins.queue = f"qPoolDynamic{qi or ''}" 
                        gath.append(xg)
                    for g0 in range(0, Tb, GRP):
                        n = min(GRP, Tb - g0)
                        xg = gath[g0 // GANG]
                        joff = g0 - (g0 // GANG) * GANG
                        # one-hot [e, dst]
                        oh = sb.tile([128, GRP * 128], fp32, tag="oh")
                        nc.vector.tensor_tensor(
                            oh[:, : n * 128].rearrange("p (t d) -> p t d", t=n),
                            dc_sb[:, g0:g0 + n, None].to_broadcast([128, n, 128]),
                            ir_sb[:, None, :].to_broadcast([128, n, 128]),
                            OP.is_equal)
                        # replicate dst_row across partitions via rank-1 matmul
                        drow_ps = ps1.tile([128, GRP * 128], fp32, tag="drow")
                        nc.tensor.matmul(drow_ps[:, : n * 128], ones_sb[:],
                                         dr_sb[:, g0 * 128:(g0 + n) * 128],
                                         start=True, stop=True)
                        # one-hot^T [dst, e]
                        ohT = sb.tile([128, GRP * 128], fp32, tag="ohT")
                        nc.vector.tensor_tensor(
                            ohT[:, : n * 128],
                            ic_sb[:].to_broadcast([128, n * 128]),
                            drow_ps[:, : n * 128],
                            OP.is_equal)
                        eT_ps = ps.tile([D, GRP * 128], fp32, tag="eT")
                        for j in range(n):
                            sl = slice(j * 128, (j + 1) * 128)
                            nc.tensor.matmul(eT_ps[:, sl], xr_sb[:],
                                             ohT[:, sl], start=True, stop=False)
                            nc.tensor.matmul(eT_ps[:, sl],
                                             xg[:, joff + j, :], ident_sb[:],
                                             start=False, stop=True)
                        # leaky(x, 0.2) = relu(0.8*x) + 0.2*x  (HW Lrelu
                        # slope is hard-wired to 0.01, so compose instead)
                        eT_sb = sb.tile([D, GRP * 128], fp32, tag="eTs")
                        nc.scalar.activation(eT_sb[:, : n * 128],
                                             eT_ps[:, : n * 128],
                                             AF.Relu, scale=0.8)
                        nc.vector.scalar_tensor_tensor(
                            eT_sb[:, : n * 128], eT_ps[:, : n * 128], 0.2,
                            eT_sb[:, : n * 128], OP.mult, OP.add)
                        al_ps = ps1.tile([128, GRP * H], fp32, tag="al")
                        for j in range(n):
                            nc.tensor.matmul(
                                al_ps[:, j * H:(j + 1) * H],
                                eT_sb[:, j * 128:(j + 1) * 128],
                                att_sb[:], start=True, stop=True)
                        msg = sb.tile([128, GRP, D + H], fp32, tag="msg")
                        nc.scalar.activation(
                            msg[:, :n, D:D + H],
                            al_ps[:, : n * H].rearrange("p (t h) -> p t h", t=n),
                            AF.Exp)
                        nc.vector.tensor_tensor(
                            msg[:, :n, :D].rearrange("p t (h c) -> p t h c", h=H),
                            xg[:, joff:joff + n, :].rearrange(
                                "p t (h c) -> p t h c", h=H),
                            msg[:, :n, D:D + H, None].to_broadcast(
                                [128, n, H, CH]),
                            OP.mult)
                        for j in range(n):
                            t = g0 + j
                            nc.tensor.matmul(
                                agg_ps[:], oh[:, j * 128:(j + 1) * 128],
                                msg[:, j, :], start=(t == 0), stop=(t == Tb - 1),
                                skip_group_check=True)
                    block_epilogue(b, agg_ps)

            # ---- layer 1 epilogue: h -> hT_dram --------------------------
            def epi1(b, agg_ps):
                den = sb.tile([128, 2], fp32, tag="den1")
                nc.vector.tensor_scalar(den[:], agg_ps[:, 128:130], 1e-16, None,
                                        OP.add)
                rcp = sb.tile([128, 2], fp32, tag="rcp1")
                nc.vector.reciprocal(rcp[:], den[:])
                h_sb = sb.tile([128, 128], fp32, tag="h")
                nc.vector.tensor_tensor(
                    h_sb[:].rearrange("p (h c) -> p h c", h=2),
                    agg_ps[:, 0:128].rearrange("p (h c) -> p h c", h=2),
                    rcp[:, :, None].to_broadcast([128, 2, 64]), OP.mult)
                if use_bias:
                    nc.vector.tensor_tensor(h_sb[:], h_sb[:], bias1_sb[:],
                                            OP.add)
                h2_sb = sb.tile([128, 128], fp32, tag="h2")
                nc.scalar.activation(h2_sb[:], h_sb[:], AF.Lrelu, alpha=0.01)
                hT_ps = ps1.tile([128, 128], fp32, tag="hT")
                nc.tensor.transpose(hT_ps[:], h2_sb[:], ident_sb[:])
                hT_sb = sb.tile([128, 128], fp32, tag="hTs")
                nc.scalar.copy(hT_sb[:], hT_ps[:])
                nc.sync.dma_start(out=hT_dram[:, b * 128:(b + 1) * 128],
                                  in_=hT_sb[:])

            edge_pass(128, 2, xl_dram[:], xr_dram, att1_sb, epi1)

            # ---- phase D: xl2/xr2 ----------------------------------------
            for i in range(NBLK):
                rows = min(128, NLOC - i * 128)
                hT_sb2 = sb.tile([128, 128], fp32, tag="hT2")
                nc.sync.dma_start(out=hT_sb2[:],
                                  in_=hT_dram[:, i * 128:(i + 1) * 128])
                p2 = ps1.tile([128, 8], fp32, tag="al")
                nc.tensor.matmul(p2[:, 0:4], hT_sb2[:], Wl2_sb[:],
                                 start=True, stop=True)
                nc.tensor.matmul(p2[:, 4:8], hT_sb2[:], Wr2_sb[:],
                                 start=True, stop=True)
                x2_sb = sb.tile([128, 8], fp32, tag="x2")
                nc.scalar.copy(x2_sb[:], p2[:])
                if use_bias:
                    nc.vector.tensor_tensor(x2_sb[:, 0:4], x2_sb[:, 0:4],
                                            bl2_sb[:], OP.add)
                    nc.vector.tensor_tensor(x2_sb[:, 4:8], x2_sb[:, 4:8],
                                            br2_sb[:], OP.add)
                nc.sync.dma_start(out=xl2_loc[i * 128:i * 128 + rows, :],
                                  in_=x2_sb[:rows, 0:4])
                nc.sync.dma_start(out=xr2_dram[i * 128:i * 128 + rows, :],
                                  in_=x2_sb[:rows, 4:8])

            # ---- AllGather xl2 ------------------------------------------
            if no_collective:  # timing-sim variant (TimelineSim can't model cc)
                nc.sync.dma_start(out=xl2_full[0:NLOC, :], in_=xl2_loc[:])
            else:
                nc.gpsimd.collective_compute(
                    "AllGather", mybir.AluOpType.bypass,
                    replica_groups=[list(range(NC))],
                    ins=[xl2_loc[:].opt()], outs=[xl2_full[:].opt()])

            # ---- layer 2 edge pass + final softmax -----------------------
            stage = sb.tile([128, NBLK, 5], fp32, tag="stage")

            def epi2(b, agg_ps):
                nc.vector.tensor_copy(stage[:, b, :], agg_ps[:, 0:5])

            edge_pass(4, 1, xl2_full[:], xr2_dram, att2_sb, epi2)

            den2 = sb.tile([128, NBLK], fp32, tag="den2")
            nc.vector.tensor_scalar(den2[:], stage[:, :, 4], 1e-16, None, OP.add)
            rcp2 = sb.tile([128, NBLK], fp32, tag="rcp2")
            nc.vector.reciprocal(rcp2[:], den2[:])
            o_sb = sb.tile([128, NBLK, 4], fp32, tag="o")
            nc.vector.tensor_tensor(o_sb[:], stage[:, :, 0:4],
                                    rcp2[:, :, None].to_broadcast([128, NBLK, 4]),
                                    OP.mult)
            if use_bias:
                nc.vector.tensor_tensor(
                    o_sb[:], o_sb[:],
                    bias2_sb[:, None, :].to_broadcast([128, NBLK, 4]), OP.add)
            mx = sb.tile([128, NBLK, 1], fp32, tag="mx")
            nc.vector.reduce_max(mx[:], o_sb[:], axis=mybir.AxisListType.X)
            nc.vector.tensor_tensor(o_sb[:], o_sb[:],
                                    mx[:].to_broadcast([128, NBLK, 4]),
                                    OP.subtract)
            ex = sb.tile([128, NBLK, 4], fp32, tag="ex")
            nc.scalar.activation(ex[:], o_sb[:], AF.Exp)
            sm = sb.tile([128, NBLK, 1], fp32, tag="sm")
            nc.vector.reduce_sum(sm[:], ex[:], axis=mybir.AxisListType.X)
            rs = sb.tile([128, NBLK, 1], fp32, tag="rs")
            nc.vector.reciprocal(rs[:], sm[:])
            nc.vector.tensor_tensor(ex[:], ex[:],
                                    rs[:].to_broadcast([128, NBLK, 4]), OP.mult)
            nc.sync.dma_start(
                out=out[0:48 * 128, :].rearrange("(b p) c -> p b c", p=128),
                in_=ex[:, 0:48, :])
            nc.sync.dma_start(out=out[48 * 128:NLOC, :], in_=ex[:106, 48, :])
    _hoist_multi_waits(nc, mybir)
    return nc


def kernel(**inputs):
    from concourse.bass_utils import run_bass_kernel_spmd

    x = np.asarray(inputs["x"], dtype=np.float32)
    src_off, dst_col, dst_row, T_b, tile0, T = _preprocess(
        np.asarray(inputs["edge_index"]))
    use_bias = any(
        np.any(np.asarray(inputs[k]) != 0)
        for k in ("bl1", "br1", "bias1", "bl2", "br2", "bias2"))
    nc = _build_program(T_b, tile0, T, use_bias)

    xT = np.zeros((128, NPAD), dtype=np.float32)
    xT[:, :N] = x.T
    att1 = np.asarray(inputs["att1"], dtype=np.float32).reshape(2, 64)
    att1_sb = np.zeros((128, 2), dtype=np.float32)
    att1_sb[0:64, 0] = att1[0]
    att1_sb[64:128, 1] = att1[1]
    att2_sb = np.asarray(inputs["att2"], dtype=np.float32).reshape(4, 1)
    f32 = lambda k: np.ravel(np.asarray(inputs[k], dtype=np.float32))
    m32 = lambda k, s: np.asarray(inputs[k], dtype=np.float32).reshape(s)
    common = dict(
        xT=xT,
        Wl1=m32("Wl1", (128, 128)), Wr1=m32("Wr1", (128, 128)), att1=att1_sb,
        Wl2=m32("Wl2", (128, 4)), Wr2=m32("Wr2", (128, 4)), att2=att2_sb,
        bl1=np.tile(f32("bl1")[None, :], (128, 1)),
        br1=np.tile(f32("br1")[None, :], (128, 1)),
        bias1r=np.tile(f32("bias1")[None, :], (128, 1)),
        bl2=np.tile(f32("bl2")[None, :], (128, 1)),
        br2=np.tile(f32("br2")[None, :], (128, 1)),
        bias2r=np.tile(f32("bias2")[None, :], (128, 1)),
        ones_row=np.ones((1, 128), dtype=np.float32),
        ident=np.eye(128, dtype=np.float32),
        iota_col=np.arange(128, dtype=np.float32)[:, None].copy(),
        iota_rep=np.tile(np.arange(128, dtype=np.float32)[None, :], (128, 1)),
    )
    in_maps = []
    for c in range(NC):
        xT_loc = np.zeros((128, NBLK * 128), dtype=np.float32)
        hi = min(N, c * NLOC + NBLK * 128)
        xT_loc[:, : hi - c * NLOC] = x.T[:, c * NLOC:hi]
        in_maps.append(dict(common, xT_loc=xT_loc, src_off=src_off[c],
                            dst_col=dst_col[c], dst_row=dst_row[c]))

    profile = os.environ.get("KERNEL_PROFILE", "0") == "1"
    res = run_bass_kernel_spmd(
        nc, in_maps, core_ids=list(range(NC)), trace=profile)
    _EXEC_NS["v"] = res.exec_time_ns
    if profile:
        print(f"HW exec time: {res.exec_time_ns} ns "
              f"(mean {res.mean_exec_time_ns})", flush=True)
    out = np.concatenate([res.results[c]["out"] for c in range(NC)], axis=0)
    return out
